# revision 1
# baseline (speedup 1.0000x reference)
import jax
import jax.numpy as jnp
import numpy as np
from functools import partial

N = 8192
IN_C = 512
OUT_C = 256
NCORES = 8
ROWS = N // NCORES  # 1024 rows per core


@partial(jax.pmap, axis_name="i", in_axes=(0, 0, None))
def _gcn_shard(adj_local, x_local, weight):
    # adj_local: [ROWS, N], x_local: [ROWS, IN_C], weight: [IN_C, OUT_C]
    core = jax.lax.axis_index("i")
    row0 = core * ROWS

    # degree of local rows (adj without self-loops), then all-gather full dinv
    deg_local = jnp.sum(adj_local, axis=1)                    # [ROWS]
    deg_full = jax.lax.all_gather(deg_local, "i").reshape(N)  # [N]
    dinv_full = jax.lax.rsqrt(deg_full)                       # [N]
    dinv_local = jax.lax.dynamic_slice(dinv_full, (row0,), (ROWS,))

    # A + I restricted to this row block
    col = jax.lax.broadcasted_iota(jnp.int32, (ROWS, N), 1)
    row = jax.lax.broadcasted_iota(jnp.int32, (ROWS, N), 0) + row0
    a_plus_i = adj_local + (col == row).astype(adj_local.dtype)

    # A_hat row block = dinv_local[:,None] * (A+I) * dinv_full[None,:]
    a_hat = dinv_local[:, None] * a_plus_i * dinv_full[None, :]

    # XW: local rows then all-gather the small [N, OUT_C] matrix
    xw_local = x_local @ weight                               # [ROWS, OUT_C]
    xw_full = jax.lax.all_gather(xw_local, "i").reshape(N, OUT_C)

    return jax.nn.relu(a_hat @ xw_full)                       # [ROWS, OUT_C]


def kernel(input, adj_matrix, weight):
    input = np.asarray(input, dtype=np.float32)
    adj_matrix = np.asarray(adj_matrix, dtype=np.float32)
    weight = np.asarray(weight, dtype=np.float32)

    adj_sh = adj_matrix.reshape(NCORES, ROWS, N)
    x_sh = input.reshape(NCORES, ROWS, IN_C)

    out = _gcn_shard(adj_sh, x_sh, weight)                    # [NCORES, ROWS, OUT_C]
    return np.asarray(out).reshape(N, OUT_C)
